# revision 4
# baseline (speedup 1.0000x reference)
"""TRN2 Bass kernel for nn_CommLayer (gnn message passing).

Math: x [B=65536, 512] viewed as [B, 8 agents, 64]; per agent a:
    y_a = tanh(x_a @ Wh.T + (sum_{a'!=a} x_{a'}) @ Wc.T / 7)
Rewritten with s = sum_a x_a:
    y_a = tanh(x_a @ WdT + s @ Wc7T),  WdT = Wh.T - Wc.T/7, Wc7T = Wc.T/7
a block-diagonal matmul plus a shared rank-64 term -- 7x less PE work
than the dense 512x512 matmul.

Everything runs in the TRANSPOSED domain in fp16; tanh output ships as
int8 (x127). Per-core traffic: 10.4 MB loads + 4.2 MB stores; at the
~360-400 GB/s the SDMA engines sustain that is ~37-40 us, the roofline
for this shape. PE (28 us), ACT tanh (31 us), DVE quant (20 us) all
fit underneath.

v4 structure (distilled from v1 @58.9, v2 @63.3, v3 @60.0 traces):
  - One [128, 5120] fp16 tile per 1024-row group: x.T chunks in cols
    0:4096, s.T pre-DUPLICATED across both partition halves by the
    host in cols 4096:5120. The matmul moving operand reads the s
    region directly: zero on-device data marshalling. (v2/v3 built the
    s-dup with 4 DVE copies per group; the Tile scheduler hoisted
    those copies ahead of the quants on DVE's stream, which inflated
    the store-dispatch semaphore targets -- stores fired 4-16 us after
    their data was ready and the whole pipeline backpressure-stalled
    into a HAM re-throttle. +1 MB of HBM beats that every time.)
  - wpk + group 0 (as s+chunks23 / chunks01 halves) load on the sync
    HWDGE ring, dispatched at t=0 (the scalar ring's first dispatch
    sits behind the framework's 1.3 us ACT table load); groups 1-7
    load on the scalar ring. Each ring saturates all 16 SDMA engines
    when alone; they round-robin fairly when both have work.
  - Every group computes half 1 (chunks 2,3) first so compute starts
    as soon as the 768 KB g0A slice lands: first tanh ~7 us.
  - 8 warmup matmuls on a memset tile ramp the PE HAM clock gate
    (1.2 -> 2.4 GHz, ~3.4 us activity window) during the load latency.
  - Stores per half ([128, 2048] int8): half 1 on the sync ring
    (FIFO behind only group 0's loads), half 0 on gpsimd SWDGE.
    Two queues overlap the per-DMA HBM write-receipt stall that
    serialized v1's store tail at ~1.15 us/store.
  - oq bufs=4 / og bufs=3 so a briefly-starved store can never
    backpressure quant -> tanh -> PE.
"""
import sys

sys.path.insert(0, "/opt/trn_rl_repo")

import numpy as np

BATCH = 65536
D = 512
NAGENT = 8
DA = 64
NORM = NAGENT - 1
NCORES = 8
SHARD = BATCH // NCORES  # 8192
R = 1024                 # rows per group
NGROUP = SHARD // R      # 8
NCHUNK = D // 128        # 4
XCOL = NCHUNK * R        # 4096 x cols per group tile
SCOL = 1024              # s-dup cols appended per group tile
GCOL = XCOL + SCOL       # 5120

_CACHE: dict = {}


def _build_nc():
    import concourse.mybir as mybir
    import concourse.tile as tile
    from concourse import bacc

    nc = bacc.Bacc("TRN2", target_bir_lowering=False, debug=False)

    f16 = mybir.dt.float16
    f32 = mybir.dt.float32
    i8 = mybir.dt.int8

    x6_d = nc.dram_tensor(
        "x6", [NGROUP * 128, GCOL], f16, kind="ExternalInput"
    )
    wpk_d = nc.dram_tensor("wpk", [128, 256], f16, kind="ExternalInput")
    y4_d = nc.dram_tensor(
        "y4", [NGROUP * 128, XCOL], i8, kind="ExternalOutput"
    )

    xv = x6_d[:].rearrange("(g p) f -> g p f", p=128)  # [8, 128, 5120]
    yv = y4_d[:].rearrange("(g p) f -> g p f", p=128)  # [8, 128, 4096]

    with tile.TileContext(nc) as tc:
        with (
            tc.tile_pool(name="const", bufs=1) as const,
            tc.tile_pool(name="xg", bufs=NGROUP) as xgp,
            tc.tile_pool(name="og", bufs=3) as ogp,
            tc.tile_pool(name="oq", bufs=4) as oqp,
            tc.tile_pool(name="psy", bufs=2, space="PSUM") as psyp,
        ):
            # ---- load issue ----
            wpk = const.tile([128, 256], f16)
            nc.sync.dma_start(wpk[:], wpk_d[:])
            wd2 = wpk[:, 0:128]
            wcs = wpk[:, 128:256]
            xg_tiles = []
            for g in range(NGROUP):
                xg = xgp.tile([128, GCOL], f16, tag="xg", name=f"xg{g}")
                if g == 0:
                    # s-dup + chunks 2,3 first: compute starts on half 1
                    nc.sync.dma_start(xg[:, 2048:GCOL], xv[g][:, 2048:GCOL])
                    nc.sync.dma_start(xg[:, 0:2048], xv[g][:, 0:2048])
                else:
                    nc.scalar.dma_start(xg[:], xv[g])
                xg_tiles.append(xg)

            # ---- PE warmup: dummy matmuls ramp the HAM clock gate
            # while group 0's load is in flight ----
            mset = const.tile([128, 512], f16)
            nc.vector.memset(mset[:], 0.0)
            psw = psyp.tile([128, 2048], f32, tag="psy", name="psy_warm")
            for w in range(8):
                nc.tensor.matmul(
                    psw[:, 0:512], mset[:, 0:128], mset[:],
                    start=True, stop=True,
                )

            for g in range(NGROUP):
                xg = xg_tiles[g]
                sgv = xg[:, XCOL:GCOL]  # s.T duplicated, [128, 1024]
                for half in (1, 0):  # half 1 first: its data arrives first
                    psy = psyp.tile([128, 2048], f32, tag="psy",
                                    name=f"psy{g}_{half}")
                    for ci in range(2):
                        co = 2 * half + ci
                        for h in range(2):
                            fs = slice(ci * R + h * 512,
                                       ci * R + (h + 1) * 512)
                            nc.tensor.matmul(
                                psy[:, fs], wcs,
                                sgv[:, h * 512:(h + 1) * 512],
                                start=True, stop=False,
                            )
                            nc.tensor.matmul(
                                psy[:, fs], wd2,
                                xg[:, co * R + h * 512:co * R + (h + 1) * 512],
                                start=False, stop=True,
                            )
                    og = ogp.tile([128, 2048], f16, tag="og",
                                  name=f"og{g}_{half}")
                    nc.scalar.activation(
                        og[:], psy[:],
                        mybir.ActivationFunctionType.Tanh,
                    )
                    oq = oqp.tile([128, 2048], i8, tag="oq",
                                  name=f"oq{g}_{half}")
                    nc.vector.tensor_scalar_mul(oq[:], og[:], 127.0)
                    dst = yv[g][:, half * 2048:(half + 1) * 2048]
                    if half:
                        nc.sync.dma_start(dst, oq[:])
                    else:
                        nc.gpsimd.dma_start(dst, oq[:])

    nc.compile()
    return nc


def _get_nc():
    if "nc" not in _CACHE:
        _CACHE["nc"] = _build_nc()
    return _CACHE["nc"]


def _prepare_in_maps(inputs) -> list[dict]:
    """Full inputs -> per-core in_maps (host does transpose + fp16 cast)."""
    x = np.asarray(inputs["x"], dtype=np.float32)
    hw = np.asarray(inputs["hidden_weights"], dtype=np.float32)
    cw = np.asarray(inputs["communication_weights"], dtype=np.float32)
    assert x.shape == (BATCH, D), x.shape

    wc7t = cw.T / np.float32(NORM)          # [64, 64]
    wdt = hw.T - wc7t                       # [64, 64]
    wpk = np.zeros((128, 256), dtype=np.float16)
    wpk[0:64, 0:64] = wdt                   # wd2 block-diagonal
    wpk[64:128, 64:128] = wdt
    wpk[0:64, 128:192] = wc7t               # wcs: wc7t in both col blocks
    wpk[0:64, 192:256] = wc7t

    s = x.reshape(BATCH, NAGENT, DA).sum(axis=1)        # [B, 64] in f32
    x16 = x.astype(np.float16)
    s16 = s.astype(np.float16)

    in_maps = []
    for i in range(NCORES):
        rows = slice(i * SHARD, (i + 1) * SHARD)
        xt = x16[rows].T                                 # [512, 8192]
        st = s16[rows].T                                 # [64, 8192]
        # x cols: [4, 128, 8, 1024] -> [8, 128, 4, 1024]
        x6 = np.empty((NGROUP, 128, GCOL), dtype=np.float16)
        x6[:, :, 0:XCOL] = (
            xt.reshape(NCHUNK, 128, NGROUP, R).transpose(2, 1, 0, 3)
            .reshape(NGROUP, 128, XCOL)
        )
        # s-dup: [g, p, r] = s.T[p % 64, g*1024 + r]
        sd = st.reshape(DA, NGROUP, R).transpose(1, 0, 2)  # [8, 64, 1024]
        x6[:, 0:64, XCOL:GCOL] = sd
        x6[:, 64:128, XCOL:GCOL] = sd
        in_maps.append({"x6": x6.reshape(NGROUP * 128, GCOL), "wpk": wpk})
    return in_maps


def _decode_out(res) -> np.ndarray:
    y = np.empty((BATCH, D), dtype=np.float32)
    inv = np.float32(1.0 / 127.0)
    for i, r in enumerate(res.results):
        y4 = r["y4"].reshape(NGROUP, 128, NCHUNK, R)
        # y4[g, p, co, r] = 127 * y[g*R + r, co*128 + p]
        yi = y4.transpose(0, 3, 2, 1).reshape(SHARD, D)
        y[i * SHARD:(i + 1) * SHARD] = yi
    y *= inv
    return y


def kernel(**inputs) -> np.ndarray:
    from concourse.bass_utils import run_bass_kernel_spmd

    nc = _get_nc()
    in_maps = _prepare_in_maps(inputs)
    res = run_bass_kernel_spmd(nc, in_maps, core_ids=list(range(NCORES)))
    return _decode_out(res)


# revision 6
# speedup vs baseline: 1.1139x; 1.1139x over previous
"""TRN2 Bass kernel for nn_CommLayer (gnn message passing).

Math: x [B=65536, 512] viewed as [B, 8 agents, 64]; per agent a:
    y_a = tanh(x_a @ Wh.T + (sum_{a'!=a} x_{a'}) @ Wc.T / 7)
Rewritten with s = sum_a x_a:
    y_a = tanh(x_a @ WdT + s @ Wc7T),  WdT = Wh.T - Wc.T/7, Wc7T = Wc.T/7
a block-diagonal matmul plus a shared rank-64 term -- 7x less PE work
than the dense 512x512 matmul.

Everything runs in the TRANSPOSED domain in fp16; tanh output ships as
int8 (x127). Per-core traffic: 10.4 MB loads + 4.2 MB stores; at the
~360-400 GB/s the SDMA engines sustain that is ~37-40 us, the roofline
for this shape. PE (28 us), ACT tanh (31 us), DVE quant (20 us) all
fit underneath.

v5 structure (distilled from v1 @58.9, v2 @63.3, v3 @60.0, v4 @68.6
traces; the v4 trace exposed the key law: a dma_start BLOCKS its
issuing sequencer whenever the HWDGE ring is full, so big load queues
must live on an engine with no other work):
  - One [128, 5120] fp16 tile per 1024-row group: x.T chunks in cols
    0:4096, s.T pre-DUPLICATED across both partition halves by the
    host in cols 4096:5120. The matmul moving operand reads the s
    region directly: zero on-device data marshalling. (v2/v3 built the
    s-dup with 4 DVE copies per group; the Tile scheduler hoisted
    those copies ahead of the quants on DVE's stream, which inflated
    the store-dispatch semaphore targets -- stores fired 4-16 us after
    their data was ready and the whole pipeline backpressure-stalled
    into a HAM re-throttle. +1 MB of HBM beats that every time.)
  - ALL loads (wpk, then group 0 as s+chunks23 / chunks01 halves,
    then groups 1-7 whole) dispatch from the sync engine, which does
    nothing else: when the qSP HWDGE ring fills, only the idle SP
    sequencer blocks. v4 put 7 of them on the ACT engine and its
    sequencer sat inside dispatch #5 until ~24 us -- after which the
    first tanh could finally issue.
  - Every group computes half 1 (chunks 2,3) first so compute starts
    as soon as the 768 KB g0A slice lands: first tanh ~7 us.
  - 8 warmup matmuls on a memset tile ramp the PE HAM clock gate
    (1.2 -> 2.4 GHz, ~3.4 us activity window) during the load latency.
  - Stores per half ([128, 2048] int8) all on gpsimd SWDGE: Q7
    emission (~0.85 us each, serialized on the otherwise-idle Pool
    engine) and the ~1.15 us/store drain+receipt pace both keep up
    with the 1.9 us/half production cadence, so stores trail
    production instead of piling into a tail.
  - oq bufs=4 / og bufs=3 so a briefly-starved store can never
    backpressure quant -> tanh -> PE.
"""
import sys

sys.path.insert(0, "/opt/trn_rl_repo")

import numpy as np

BATCH = 65536
D = 512
NAGENT = 8
DA = 64
NORM = NAGENT - 1
NCORES = 8
SHARD = BATCH // NCORES  # 8192
R = 1024                 # rows per group
NGROUP = SHARD // R      # 8
NCHUNK = D // 128        # 4
XCOL = NCHUNK * R        # 4096 x cols per group tile
SCOL = 1024              # s-dup cols appended per group tile
GCOL = XCOL + SCOL       # 5120

_CACHE: dict = {}


def _build_nc():
    import concourse.mybir as mybir
    import concourse.tile as tile
    from concourse import bacc

    nc = bacc.Bacc("TRN2", target_bir_lowering=False, debug=False)

    f16 = mybir.dt.float16
    f32 = mybir.dt.float32
    i8 = mybir.dt.int8

    x6_d = nc.dram_tensor(
        "x6", [NGROUP * 128, GCOL], f16, kind="ExternalInput"
    )
    wpk_d = nc.dram_tensor("wpk", [128, 256], f16, kind="ExternalInput")
    y4_d = nc.dram_tensor(
        "y4", [NGROUP * 128, XCOL], i8, kind="ExternalOutput"
    )

    xv = x6_d[:].rearrange("(g p) f -> g p f", p=128)  # [8, 128, 5120]
    yv = y4_d[:].rearrange("(g p) f -> g p f", p=128)  # [8, 128, 4096]

    with tile.TileContext(nc) as tc:
        with (
            tc.tile_pool(name="const", bufs=1) as const,
            tc.tile_pool(name="xg", bufs=NGROUP) as xgp,
            tc.tile_pool(name="og", bufs=3) as ogp,
            tc.tile_pool(name="oq", bufs=4) as oqp,
            tc.tile_pool(name="psy", bufs=2, space="PSUM") as psyp,
        ):
            # ---- load issue ----
            wpk = const.tile([128, 256], f16)
            nc.sync.dma_start(wpk[:], wpk_d[:])
            wd2 = wpk[:, 0:128]
            wcs = wpk[:, 128:256]
            xg_tiles = []
            for g in range(NGROUP):
                xg = xgp.tile([128, GCOL], f16, tag="xg", name=f"xg{g}")
                if g == 0:
                    # s-dup + chunks 2,3 first: compute starts on half 1
                    nc.sync.dma_start(xg[:, 2048:GCOL], xv[g][:, 2048:GCOL])
                    nc.sync.dma_start(xg[:, 0:2048], xv[g][:, 0:2048])
                else:
                    nc.sync.dma_start(xg[:], xv[g])
                xg_tiles.append(xg)

            # ---- PE warmup: dummy matmuls ramp the HAM clock gate
            # while group 0's load is in flight ----
            mset = const.tile([128, 512], f16)
            nc.vector.memset(mset[:], 0.0)
            psw = psyp.tile([128, 2048], f32, tag="psy", name="psy_warm")
            for w in range(8):
                nc.tensor.matmul(
                    psw[:, 0:512], mset[:, 0:128], mset[:],
                    start=True, stop=True,
                )

            for g in range(NGROUP):
                xg = xg_tiles[g]
                sgv = xg[:, XCOL:GCOL]  # s.T duplicated, [128, 1024]
                for half in (1, 0):  # half 1 first: its data arrives first
                    psy = psyp.tile([128, 2048], f32, tag="psy",
                                    name=f"psy{g}_{half}")
                    for ci in range(2):
                        co = 2 * half + ci
                        for h in range(2):
                            fs = slice(ci * R + h * 512,
                                       ci * R + (h + 1) * 512)
                            nc.tensor.matmul(
                                psy[:, fs], wcs,
                                sgv[:, h * 512:(h + 1) * 512],
                                start=True, stop=False,
                            )
                            nc.tensor.matmul(
                                psy[:, fs], wd2,
                                xg[:, co * R + h * 512:co * R + (h + 1) * 512],
                                start=False, stop=True,
                            )
                    og = ogp.tile([128, 2048], f16, tag="og",
                                  name=f"og{g}_{half}")
                    nc.scalar.activation(
                        og[:], psy[:],
                        mybir.ActivationFunctionType.Tanh,
                    )
                    oq = oqp.tile([128, 2048], i8, tag="oq",
                                  name=f"oq{g}_{half}")
                    nc.vector.tensor_scalar_mul(oq[:], og[:], 127.0)
                    dst = yv[g][:, half * 2048:(half + 1) * 2048]
                    nc.gpsimd.dma_start(dst, oq[:])

    nc.compile()
    return nc


def _get_nc():
    if "nc" not in _CACHE:
        _CACHE["nc"] = _build_nc()
    return _CACHE["nc"]


def _prepare_in_maps(inputs) -> list[dict]:
    """Full inputs -> per-core in_maps (host does transpose + fp16 cast)."""
    x = np.asarray(inputs["x"], dtype=np.float32)
    hw = np.asarray(inputs["hidden_weights"], dtype=np.float32)
    cw = np.asarray(inputs["communication_weights"], dtype=np.float32)
    assert x.shape == (BATCH, D), x.shape

    wc7t = cw.T / np.float32(NORM)          # [64, 64]
    wdt = hw.T - wc7t                       # [64, 64]
    wpk = np.zeros((128, 256), dtype=np.float16)
    wpk[0:64, 0:64] = wdt                   # wd2 block-diagonal
    wpk[64:128, 64:128] = wdt
    wpk[0:64, 128:192] = wc7t               # wcs: wc7t in both col blocks
    wpk[0:64, 192:256] = wc7t

    s = x.reshape(BATCH, NAGENT, DA).sum(axis=1)        # [B, 64] in f32
    x16 = x.astype(np.float16)
    s16 = s.astype(np.float16)

    in_maps = []
    for i in range(NCORES):
        rows = slice(i * SHARD, (i + 1) * SHARD)
        xt = x16[rows].T                                 # [512, 8192]
        st = s16[rows].T                                 # [64, 8192]
        # x cols: [4, 128, 8, 1024] -> [8, 128, 4, 1024]
        x6 = np.empty((NGROUP, 128, GCOL), dtype=np.float16)
        x6[:, :, 0:XCOL] = (
            xt.reshape(NCHUNK, 128, NGROUP, R).transpose(2, 1, 0, 3)
            .reshape(NGROUP, 128, XCOL)
        )
        # s-dup: [g, p, r] = s.T[p % 64, g*1024 + r]
        sd = st.reshape(DA, NGROUP, R).transpose(1, 0, 2)  # [8, 64, 1024]
        x6[:, 0:64, XCOL:GCOL] = sd
        x6[:, 64:128, XCOL:GCOL] = sd
        in_maps.append({"x6": x6.reshape(NGROUP * 128, GCOL), "wpk": wpk})
    return in_maps


def _decode_out(res) -> np.ndarray:
    y = np.empty((BATCH, D), dtype=np.float32)
    inv = np.float32(1.0 / 127.0)
    for i, r in enumerate(res.results):
        y4 = r["y4"].reshape(NGROUP, 128, NCHUNK, R)
        # y4[g, p, co, r] = 127 * y[g*R + r, co*128 + p]
        yi = y4.transpose(0, 3, 2, 1).reshape(SHARD, D)
        y[i * SHARD:(i + 1) * SHARD] = yi
    y *= inv
    return y


def kernel(**inputs) -> np.ndarray:
    from concourse.bass_utils import run_bass_kernel_spmd

    nc = _get_nc()
    in_maps = _prepare_in_maps(inputs)
    res = run_bass_kernel_spmd(nc, in_maps, core_ids=list(range(NCORES)))
    return _decode_out(res)


# revision 7
# speedup vs baseline: 1.2260x; 1.1006x over previous
"""TRN2 Bass kernel for nn_CommLayer (gnn message passing).

Math: x [B=65536, 512] viewed as [B, 8 agents, 64]; per agent a:
    y_a = tanh(x_a @ Wh.T + (sum_{a'!=a} x_{a'}) @ Wc.T / 7)
Rewritten with s = sum_a x_a:
    y_a = tanh(x_a @ WdT + s @ Wc7T),  WdT = Wh.T - Wc.T/7, Wc7T = Wc.T/7
a block-diagonal matmul plus a shared rank-64 term -- 7x less PE work
than the dense 512x512 matmul.

Everything runs in the TRANSPOSED domain in fp16; tanh output ships as
int8 (x127). Per-core traffic: 9.4 MB loads + 4.2 MB stores = 13.6 MB;
the 16 SDMA engines sustain ~400 GB/s aggregate (trace-measured), so
~34 us of saturated DMA is the roofline. PE (28 us), ACT tanh (31 us),
DVE quant (20 us) all fit underneath.

The shared-term trick that makes the layout free: the wcs stationary
only has weights in rows 0:64 (the moving partitions 64:128 multiply
zeros), so ONE [128, 512] s-pack block per group carries s.T[:, 0:512]
in partitions 0:64 and s.T[:, 512:1024] in partitions 64:128, and two
stationaries (wcs_lo / wcs_hi, weights in the lower/upper 64 rows)
select the half. Full k=128 matmuls (the PE HAM clock gate throttles
half-array work), no on-device s marshalling, no duplicated bytes.

Scheduling structure (distilled from five traced iterations: v1
@58.9us ... v5 @61.5us):
  - One [128, 4608] fp16 tile per 1024-row group (x.T chunks + the
    s-pack), loaded in ONE ~1.1 MB DMA: 9 KB/partition descriptors run
    each SDMA engine at its ~25 GB/s line rate.
  - ALL loads dispatch from the sync engine, which does nothing else:
    a dma_start BLOCKS its issuing sequencer while the HWDGE ring is
    full (~6 entries), so big load queues must live on an engine with
    no other work (v4 loaded from the ACT engine; its sequencer sat
    inside dispatch #7 until 20 us and the first tanh ran at 24).
  - Group 0 loads as s+chunks23 / chunks01 halves and every group
    computes half 1 first, so compute starts as soon as the first
    640 KB lands.
  - 8 warmup matmuls on a memset tile ramp the PE HAM clock gate
    (1.2 -> 2.4 GHz, ~3.4 us activity window) during the load latency;
    an idle gap > ~3.4 us mid-kernel re-throttles the PE and the cold
    2x matmul slowdown cascades (v2/v3 lost 5+ us to this).
  - Stores per group ([128, 4096] int8, 512 KB) on gpsimd SWDGE: Q7
    emission (~0.85 us, on the otherwise-idle Pool engine) overlaps
    the drain; ~1.7 us/store beats the 3.8 us/group production pace.
  - og bufs=4 / oq bufs=4 so the store path can lag ~2 groups without
    backpressuring quant -> tanh -> PE (v5 coupled them at depth 2 and
    the tail ran at single-store pace).
"""
import sys

sys.path.insert(0, "/opt/trn_rl_repo")

import numpy as np

BATCH = 65536
D = 512
NAGENT = 8
DA = 64
NORM = NAGENT - 1
NCORES = 8
SHARD = BATCH // NCORES  # 8192
R = 1024                 # rows per group
NGROUP = SHARD // R      # 8
NCHUNK = D // 128        # 4
XCOL = NCHUNK * R        # 4096 x cols per group tile
SCOL = 512               # s-pack cols appended per group tile
GCOL = XCOL + SCOL       # 4608

_CACHE: dict = {}


def _build_nc():
    import concourse.mybir as mybir
    import concourse.tile as tile
    from concourse import bacc

    nc = bacc.Bacc("TRN2", target_bir_lowering=False, debug=False)

    f16 = mybir.dt.float16
    f32 = mybir.dt.float32
    i8 = mybir.dt.int8

    x7_d = nc.dram_tensor(
        "x7", [NGROUP * 128, GCOL], f16, kind="ExternalInput"
    )
    wpk_d = nc.dram_tensor("wpk", [128, 384], f16, kind="ExternalInput")
    y4_d = nc.dram_tensor(
        "y4", [NGROUP * 128, XCOL], i8, kind="ExternalOutput"
    )

    xv = x7_d[:].rearrange("(g p) f -> g p f", p=128)  # [8, 128, 4608]
    yv = y4_d[:].rearrange("(g p) f -> g p f", p=128)  # [8, 128, 4096]

    with tile.TileContext(nc) as tc:
        with (
            tc.tile_pool(name="const", bufs=1) as const,
            tc.tile_pool(name="xg", bufs=NGROUP) as xgp,
            tc.tile_pool(name="og", bufs=4) as ogp,
            tc.tile_pool(name="oq", bufs=4) as oqp,
            tc.tile_pool(name="psy", bufs=2, space="PSUM") as psyp,
        ):
            # ---- load issue (all on the sync engine / qSP HWDGE) ----
            wpk = const.tile([128, 384], f16)
            nc.sync.dma_start(wpk[:], wpk_d[:])
            wd2 = wpk[:, 0:128]
            wcs = (wpk[:, 128:256], wpk[:, 256:384])  # (lo, hi) by h-slice
            xg_tiles = []
            for g in range(NGROUP):
                xg = xgp.tile([128, GCOL], f16, tag="xg", name=f"xg{g}")
                if g == 0:
                    # s-pack + chunks 2,3 first: compute starts on half 1
                    nc.sync.dma_start(xg[:, 2048:GCOL], xv[g][:, 2048:GCOL])
                    nc.sync.dma_start(xg[:, 0:2048], xv[g][:, 0:2048])
                else:
                    nc.sync.dma_start(xg[:], xv[g])
                xg_tiles.append(xg)

            # ---- PE warmup: dummy matmuls ramp the HAM clock gate
            # while group 0's load is in flight ----
            mset = const.tile([128, 512], f16)
            nc.vector.memset(mset[:], 0.0)
            psw = psyp.tile([128, 2048], f32, tag="psy", name="psy_warm")
            for w in range(8):
                nc.tensor.matmul(
                    psw[:, 0:512], mset[:, 0:128], mset[:],
                    start=True, stop=True,
                )

            for g in range(NGROUP):
                xg = xg_tiles[g]
                spk = xg[:, XCOL:GCOL]  # [128, 512] packed s.T halves
                oq = oqp.tile([128, XCOL], i8, tag="oq", name=f"oq{g}")
                for half in (1, 0):  # half 1 first: its data arrives first
                    psy = psyp.tile([128, 2048], f32, tag="psy",
                                    name=f"psy{g}_{half}")
                    for ci in range(2):
                        co = 2 * half + ci
                        for h in range(2):
                            fs = slice(ci * R + h * 512,
                                       ci * R + (h + 1) * 512)
                            nc.tensor.matmul(
                                psy[:, fs], wcs[h], spk,
                                start=True, stop=False,
                            )
                            nc.tensor.matmul(
                                psy[:, fs], wd2,
                                xg[:, co * R + h * 512:co * R + (h + 1) * 512],
                                start=False, stop=True,
                            )
                    og = ogp.tile([128, 2048], f16, tag="og",
                                  name=f"og{g}_{half}")
                    nc.scalar.activation(
                        og[:], psy[:],
                        mybir.ActivationFunctionType.Tanh,
                    )
                    nc.vector.tensor_scalar_mul(
                        oq[:, half * 2048:(half + 1) * 2048], og[:], 127.0
                    )
                nc.gpsimd.dma_start(yv[g], oq[:])

    nc.compile()
    return nc


def _get_nc():
    if "nc" not in _CACHE:
        _CACHE["nc"] = _build_nc()
    return _CACHE["nc"]


def _prepare_in_maps(inputs) -> list[dict]:
    """Full inputs -> per-core in_maps (host does transpose + fp16 cast)."""
    x = np.asarray(inputs["x"], dtype=np.float32)
    hw = np.asarray(inputs["hidden_weights"], dtype=np.float32)
    cw = np.asarray(inputs["communication_weights"], dtype=np.float32)
    assert x.shape == (BATCH, D), x.shape

    wc7t = cw.T / np.float32(NORM)          # [64, 64]
    wdt = hw.T - wc7t                       # [64, 64]
    wpk = np.zeros((128, 384), dtype=np.float16)
    wpk[0:64, 0:64] = wdt                   # wd2 block-diagonal
    wpk[64:128, 64:128] = wdt
    wpk[0:64, 128:192] = wc7t               # wcs_lo: weights in rows 0:64
    wpk[0:64, 192:256] = wc7t
    wpk[64:128, 256:320] = wc7t             # wcs_hi: weights in rows 64:128
    wpk[64:128, 320:384] = wc7t

    s = x.reshape(BATCH, NAGENT, DA).sum(axis=1)        # [B, 64] in f32
    x16 = x.astype(np.float16)
    s16 = s.astype(np.float16)

    in_maps = []
    for i in range(NCORES):
        rows = slice(i * SHARD, (i + 1) * SHARD)
        xt = x16[rows].T                                 # [512, 8192]
        st = s16[rows].T                                 # [64, 8192]
        # x cols: [4, 128, 8, 1024] -> [8, 128, 4, 1024]
        x7 = np.empty((NGROUP, 128, GCOL), dtype=np.float16)
        x7[:, :, 0:XCOL] = (
            xt.reshape(NCHUNK, 128, NGROUP, R).transpose(2, 1, 0, 3)
            .reshape(NGROUP, 128, XCOL)
        )
        # s-pack: [g, h*64 + k, c] = s.T[k, g*1024 + h*512 + c]
        x7[:, :, XCOL:GCOL] = (
            st.reshape(DA, NGROUP, 2, 512).transpose(1, 2, 0, 3)
            .reshape(NGROUP, 128, SCOL)
        )
        in_maps.append({"x7": x7.reshape(NGROUP * 128, GCOL), "wpk": wpk})
    return in_maps


def _decode_out(res) -> np.ndarray:
    y = np.empty((BATCH, D), dtype=np.float32)
    inv = np.float32(1.0 / 127.0)
    for i, r in enumerate(res.results):
        y4 = r["y4"].reshape(NGROUP, 128, NCHUNK, R)
        # y4[g, p, co, r] = 127 * y[g*R + r, co*128 + p]
        yi = y4.transpose(0, 3, 2, 1).reshape(SHARD, D)
        y[i * SHARD:(i + 1) * SHARD] = yi
    y *= inv
    return y


def kernel(**inputs) -> np.ndarray:
    from concourse.bass_utils import run_bass_kernel_spmd

    nc = _get_nc()
    in_maps = _prepare_in_maps(inputs)
    res = run_bass_kernel_spmd(nc, in_maps, core_ids=list(range(NCORES)))
    return _decode_out(res)


# revision 8
# speedup vs baseline: 1.2715x; 1.0371x over previous
"""TRN2 Bass kernel for nn_CommLayer (gnn message passing).

Math: x [B=65536, 512] viewed as [B, 8 agents, 64]; per agent a:
    y_a = tanh(x_a @ Wh.T + (sum_{a'!=a} x_{a'}) @ Wc.T / 7)
Rewritten with s = sum_a x_a:
    y_a = tanh(x_a @ WdT + s @ Wc7T),  WdT = Wh.T - Wc.T/7, Wc7T = Wc.T/7
a block-diagonal matmul plus a shared rank-64 term -- 7x less PE work
than the dense 512x512 matmul.

Everything runs in the TRANSPOSED domain in fp16; tanh output ships as
int8 (x127). Per-core traffic: 9.4 MB loads + 4.2 MB stores = 13.6 MB;
the 16 SDMA engines sustain ~400 GB/s aggregate (trace-measured), so
~34 us of saturated DMA is the roofline. PE (28 us), ACT tanh (31 us),
DVE quant (20 us) all fit underneath.

The shared-term trick that makes the layout free: the wcs stationary
only has weights in rows 0:64 (the moving partitions 64:128 multiply
zeros), so ONE [128, 512] s-pack block per group carries s.T[:, 0:512]
in partitions 0:64 and s.T[:, 512:1024] in partitions 64:128, and two
stationaries (wcs_lo / wcs_hi, weights in the lower/upper 64 rows)
select the half. Full k=128 matmuls (the PE HAM clock gate throttles
half-array work), no on-device s marshalling, no duplicated bytes.

Scheduling structure (distilled from five traced iterations: v1
@58.9us ... v5 @61.5us):
  - One [128, 4608] fp16 tile per 1024-row group (x.T chunks + the
    s-pack), loaded in ONE ~1.1 MB DMA: 9 KB/partition descriptors run
    each SDMA engine at its ~25 GB/s line rate.
  - ALL loads dispatch from the sync engine, which does nothing else:
    a dma_start BLOCKS its issuing sequencer while the HWDGE ring is
    full (~6 entries), so big load queues must live on an engine with
    no other work (v4 loaded from the ACT engine; its sequencer sat
    inside dispatch #7 until 20 us and the first tanh ran at 24).
  - Group 0 loads as s+chunks23 / chunks01 halves and every group
    computes half 1 first, so compute starts as soon as the first
    640 KB lands.
  - 5 warmup matmuls on a memset tile ramp the PE HAM clock gate
    (1.2 -> 2.4 GHz, ~3.4 us activity window) during the load latency;
    an idle gap > ~3.4 us mid-kernel re-throttles the PE and the cold
    2x matmul slowdown cascades (v2/v3 lost 5+ us to this).
  - Stores per group ([128, 4096] int8, 512 KB) dispatch from the
    sync engine too, AFTER all the loads: the qSP ring is FIFO, so no
    store byte moves until every load byte has -- loads run the SDMA
    engines at full rate and the ACT cadence is never load-starved
    (in v6 stores stole ~25% of the engines mid-run and ACT stalled
    ~1 us per group). Store g's dispatch waits on quant g, which is
    long done by the time the ring reaches it; the whole store phase
    then drains at HWDGE pace, production-gated only for the last
    2-3 groups.
  - og bufs=4 / oq bufs=5: oq must cover from quant g until store g
    completes behind the full load queue (~15 us for group 0).
"""
import sys

sys.path.insert(0, "/opt/trn_rl_repo")

import numpy as np

BATCH = 65536
D = 512
NAGENT = 8
DA = 64
NORM = NAGENT - 1
NCORES = 8
SHARD = BATCH // NCORES  # 8192
R = 1024                 # rows per group
NGROUP = SHARD // R      # 8
NCHUNK = D // 128        # 4
XCOL = NCHUNK * R        # 4096 x cols per group tile
SCOL = 512               # s-pack cols appended per group tile
GCOL = XCOL + SCOL       # 4608

_CACHE: dict = {}


def _build_nc():
    import concourse.mybir as mybir
    import concourse.tile as tile
    from concourse import bacc

    nc = bacc.Bacc("TRN2", target_bir_lowering=False, debug=False)

    f16 = mybir.dt.float16
    f32 = mybir.dt.float32
    i8 = mybir.dt.int8

    x7_d = nc.dram_tensor(
        "x7", [NGROUP * 128, GCOL], f16, kind="ExternalInput"
    )
    wpk_d = nc.dram_tensor("wpk", [128, 384], f16, kind="ExternalInput")
    y4_d = nc.dram_tensor(
        "y4", [NGROUP * 128, XCOL], i8, kind="ExternalOutput"
    )

    xv = x7_d[:].rearrange("(g p) f -> g p f", p=128)  # [8, 128, 4608]
    yv = y4_d[:].rearrange("(g p) f -> g p f", p=128)  # [8, 128, 4096]

    with tile.TileContext(nc) as tc:
        with (
            tc.tile_pool(name="const", bufs=1) as const,
            tc.tile_pool(name="xg", bufs=NGROUP) as xgp,
            tc.tile_pool(name="og", bufs=4) as ogp,
            tc.tile_pool(name="oq", bufs=5) as oqp,
            tc.tile_pool(name="psy", bufs=2, space="PSUM") as psyp,
        ):
            # ---- load issue (all on the sync engine / qSP HWDGE) ----
            wpk = const.tile([128, 384], f16)
            nc.sync.dma_start(wpk[:], wpk_d[:])
            wd2 = wpk[:, 0:128]
            wcs = (wpk[:, 128:256], wpk[:, 256:384])  # (lo, hi) by h-slice
            xg_tiles = []
            for g in range(NGROUP):
                xg = xgp.tile([128, GCOL], f16, tag="xg", name=f"xg{g}")
                if g == 0:
                    # s-pack + chunks 2,3 first: compute starts on half 1
                    nc.sync.dma_start(xg[:, 2048:GCOL], xv[g][:, 2048:GCOL])
                    nc.sync.dma_start(xg[:, 0:2048], xv[g][:, 0:2048])
                else:
                    nc.sync.dma_start(xg[:], xv[g])
                xg_tiles.append(xg)

            # ---- PE warmup: dummy matmuls ramp the HAM clock gate
            # while group 0's load is in flight ----
            mset = const.tile([128, 512], f16)
            nc.vector.memset(mset[:], 0.0)
            psw = psyp.tile([128, 2048], f32, tag="psy", name="psy_warm")
            for w in range(5):
                nc.tensor.matmul(
                    psw[:, 0:512], mset[:, 0:128], mset[:],
                    start=True, stop=True,
                )

            for g in range(NGROUP):
                xg = xg_tiles[g]
                spk = xg[:, XCOL:GCOL]  # [128, 512] packed s.T halves
                oq = oqp.tile([128, XCOL], i8, tag="oq", name=f"oq{g}")
                for half in (1, 0):  # half 1 first: its data arrives first
                    psy = psyp.tile([128, 2048], f32, tag="psy",
                                    name=f"psy{g}_{half}")
                    for ci in range(2):
                        co = 2 * half + ci
                        for h in range(2):
                            fs = slice(ci * R + h * 512,
                                       ci * R + (h + 1) * 512)
                            nc.tensor.matmul(
                                psy[:, fs], wcs[h], spk,
                                start=True, stop=False,
                            )
                            nc.tensor.matmul(
                                psy[:, fs], wd2,
                                xg[:, co * R + h * 512:co * R + (h + 1) * 512],
                                start=False, stop=True,
                            )
                    og = ogp.tile([128, 2048], f16, tag="og",
                                  name=f"og{g}_{half}")
                    nc.scalar.activation(
                        og[:], psy[:],
                        mybir.ActivationFunctionType.Tanh,
                    )
                    nc.vector.tensor_scalar_mul(
                        oq[:, half * 2048:(half + 1) * 2048], og[:], 127.0
                    )
                nc.sync.dma_start(yv[g], oq[:])

    nc.compile()
    return nc


def _get_nc():
    if "nc" not in _CACHE:
        _CACHE["nc"] = _build_nc()
    return _CACHE["nc"]


def _prepare_in_maps(inputs) -> list[dict]:
    """Full inputs -> per-core in_maps (host does transpose + fp16 cast)."""
    x = np.asarray(inputs["x"], dtype=np.float32)
    hw = np.asarray(inputs["hidden_weights"], dtype=np.float32)
    cw = np.asarray(inputs["communication_weights"], dtype=np.float32)
    assert x.shape == (BATCH, D), x.shape

    wc7t = cw.T / np.float32(NORM)          # [64, 64]
    wdt = hw.T - wc7t                       # [64, 64]
    wpk = np.zeros((128, 384), dtype=np.float16)
    wpk[0:64, 0:64] = wdt                   # wd2 block-diagonal
    wpk[64:128, 64:128] = wdt
    wpk[0:64, 128:192] = wc7t               # wcs_lo: weights in rows 0:64
    wpk[0:64, 192:256] = wc7t
    wpk[64:128, 256:320] = wc7t             # wcs_hi: weights in rows 64:128
    wpk[64:128, 320:384] = wc7t

    s = x.reshape(BATCH, NAGENT, DA).sum(axis=1)        # [B, 64] in f32
    x16 = x.astype(np.float16)
    s16 = s.astype(np.float16)

    in_maps = []
    for i in range(NCORES):
        rows = slice(i * SHARD, (i + 1) * SHARD)
        xt = x16[rows].T                                 # [512, 8192]
        st = s16[rows].T                                 # [64, 8192]
        # x cols: [4, 128, 8, 1024] -> [8, 128, 4, 1024]
        x7 = np.empty((NGROUP, 128, GCOL), dtype=np.float16)
        x7[:, :, 0:XCOL] = (
            xt.reshape(NCHUNK, 128, NGROUP, R).transpose(2, 1, 0, 3)
            .reshape(NGROUP, 128, XCOL)
        )
        # s-pack: [g, h*64 + k, c] = s.T[k, g*1024 + h*512 + c]
        x7[:, :, XCOL:GCOL] = (
            st.reshape(DA, NGROUP, 2, 512).transpose(1, 2, 0, 3)
            .reshape(NGROUP, 128, SCOL)
        )
        in_maps.append({"x7": x7.reshape(NGROUP * 128, GCOL), "wpk": wpk})
    return in_maps


def _decode_out(res) -> np.ndarray:
    y = np.empty((BATCH, D), dtype=np.float32)
    inv = np.float32(1.0 / 127.0)
    for i, r in enumerate(res.results):
        y4 = r["y4"].reshape(NGROUP, 128, NCHUNK, R)
        # y4[g, p, co, r] = 127 * y[g*R + r, co*128 + p]
        yi = y4.transpose(0, 3, 2, 1).reshape(SHARD, D)
        y[i * SHARD:(i + 1) * SHARD] = yi
    y *= inv
    return y


def kernel(**inputs) -> np.ndarray:
    from concourse.bass_utils import run_bass_kernel_spmd

    nc = _get_nc()
    in_maps = _prepare_in_maps(inputs)
    res = run_bass_kernel_spmd(nc, in_maps, core_ids=list(range(NCORES)))
    return _decode_out(res)


# revision 9
# speedup vs baseline: 1.2936x; 1.0174x over previous
"""TRN2 Bass kernel for nn_CommLayer (gnn message passing).

Math: x [B=65536, 512] viewed as [B, 8 agents, 64]; per agent a:
    y_a = tanh(x_a @ Wh.T + (sum_{a'!=a} x_{a'}) @ Wc.T / 7)
Rewritten with s = sum_a x_a:
    y_a = tanh(x_a @ WdT + s @ Wc7T),  WdT = Wh.T - Wc.T/7, Wc7T = Wc.T/7
a block-diagonal matmul plus a shared rank-64 term -- 7x less PE work
than the dense 512x512 matmul.

Everything runs in the TRANSPOSED domain in fp16; tanh output ships as
int8 (x127). Per-core traffic: 9.4 MB loads + 4.2 MB stores = 13.6 MB;
the 16 SDMA engines sustain ~400 GB/s aggregate (trace-measured), so
~34 us of saturated DMA is the roofline. PE (28 us), ACT tanh (31 us),
DVE quant (20 us) all fit underneath.

The shared-term trick that makes the layout free: the wcs stationary
only has weights in rows 0:64 (the moving partitions 64:128 multiply
zeros), so ONE [128, 512] s-pack block per group carries s.T[:, 0:512]
in partitions 0:64 and s.T[:, 512:1024] in partitions 64:128, and two
stationaries (wcs_lo / wcs_hi, weights in the lower/upper 64 rows)
select the half. Full k=128 matmuls (the PE HAM clock gate throttles
half-array work), no on-device s marshalling, no duplicated bytes.

Scheduling structure (distilled from five traced iterations: v1
@58.9us ... v5 @61.5us):
  - One [128, 4608] fp16 tile per 1024-row group (x.T chunks + the
    s-pack), loaded in ONE ~1.1 MB DMA: 9 KB/partition descriptors run
    each SDMA engine at its ~25 GB/s line rate.
  - ALL loads dispatch from the sync engine, which does nothing else:
    a dma_start BLOCKS its issuing sequencer while the HWDGE ring is
    full (~6 entries), so big load queues must live on an engine with
    no other work (v4 loaded from the ACT engine; its sequencer sat
    inside dispatch #7 until 20 us and the first tanh ran at 24).
  - Group 0 loads as s+chunks23 / chunks01 halves and every group
    computes half 1 first, so compute starts as soon as the first
    640 KB lands.
  - 10 warmup matmuls on a memset tile ramp the PE HAM clock gate
    (1.2 -> 2.4 GHz, ~3.4 us activity window) during the load latency;
    an idle gap > ~3.4 us mid-kernel re-throttles the PE and the cold
    2x matmul slowdown cascades (v2/v3 lost 5+ us to this).
  - Stores per half ([128, 2048] int8, 256 KB) dispatch from the
    sync engine too, AFTER all the loads: the qSP ring is FIFO, so no
    store byte moves until every load byte has -- loads run the SDMA
    engines at full rate and the ACT cadence is never load-starved
    (in v6 stores stole ~25% of the engines mid-run and ACT stalled
    ~1 us per group). Store g's dispatch waits on quant g, which is
    long done by the time the ring reaches it; the whole store phase
    then drains at HWDGE pace, production-gated only for the last
    2-3 groups.
  - og bufs=4 / oq (per-half) bufs=8: oq must cover from quant until
    its store completes behind the full load queue (~15 us early on).
"""
import sys

sys.path.insert(0, "/opt/trn_rl_repo")

import numpy as np

BATCH = 65536
D = 512
NAGENT = 8
DA = 64
NORM = NAGENT - 1
NCORES = 8
SHARD = BATCH // NCORES  # 8192
R = 1024                 # rows per group
NGROUP = SHARD // R      # 8
NCHUNK = D // 128        # 4
XCOL = NCHUNK * R        # 4096 x cols per group tile
SCOL = 512               # s-pack cols appended per group tile
GCOL = XCOL + SCOL       # 4608

_CACHE: dict = {}


def _build_nc():
    import concourse.mybir as mybir
    import concourse.tile as tile
    from concourse import bacc

    nc = bacc.Bacc("TRN2", target_bir_lowering=False, debug=False)

    f16 = mybir.dt.float16
    f32 = mybir.dt.float32
    i8 = mybir.dt.int8

    x7_d = nc.dram_tensor(
        "x7", [NGROUP * 128, GCOL], f16, kind="ExternalInput"
    )
    wpk_d = nc.dram_tensor("wpk", [128, 384], f16, kind="ExternalInput")
    y4_d = nc.dram_tensor(
        "y4", [NGROUP * 128, XCOL], i8, kind="ExternalOutput"
    )

    xv = x7_d[:].rearrange("(g p) f -> g p f", p=128)  # [8, 128, 4608]
    yv = y4_d[:].rearrange("(g p) f -> g p f", p=128)  # [8, 128, 4096]

    with tile.TileContext(nc) as tc:
        with (
            tc.tile_pool(name="const", bufs=1) as const,
            tc.tile_pool(name="xg", bufs=NGROUP) as xgp,
            tc.tile_pool(name="og", bufs=4) as ogp,
            tc.tile_pool(name="oq", bufs=8) as oqp,
            tc.tile_pool(name="psy", bufs=2, space="PSUM") as psyp,
        ):
            # ---- load issue (all on the sync engine / qSP HWDGE) ----
            wpk = const.tile([128, 384], f16)
            nc.sync.dma_start(wpk[:], wpk_d[:])
            wd2 = wpk[:, 0:128]
            wcs = (wpk[:, 128:256], wpk[:, 256:384])  # (lo, hi) by h-slice
            xg_tiles = []
            for g in range(NGROUP):
                xg = xgp.tile([128, GCOL], f16, tag="xg", name=f"xg{g}")
                if g == 0:
                    # s-pack + chunks 2,3 first: compute starts on half 1
                    nc.sync.dma_start(xg[:, 2048:GCOL], xv[g][:, 2048:GCOL])
                    nc.sync.dma_start(xg[:, 0:2048], xv[g][:, 0:2048])
                else:
                    nc.sync.dma_start(xg[:], xv[g])
                xg_tiles.append(xg)

            # ---- PE warmup: dummy matmuls ramp the HAM clock gate
            # while group 0's load is in flight ----
            mset = const.tile([128, 512], f16)
            nc.vector.memset(mset[:], 0.0)
            psw = psyp.tile([128, 2048], f32, tag="psy", name="psy_warm")
            for w in range(10):
                nc.tensor.matmul(
                    psw[:, 0:512], mset[:, 0:128], mset[:],
                    start=True, stop=True,
                )

            for g in range(NGROUP):
                xg = xg_tiles[g]
                spk = xg[:, XCOL:GCOL]  # [128, 512] packed s.T halves
                for half in (1, 0):  # half 1 first: its data arrives first
                    psy = psyp.tile([128, 2048], f32, tag="psy",
                                    name=f"psy{g}_{half}")
                    for ci in range(2):
                        co = 2 * half + ci
                        for h in range(2):
                            fs = slice(ci * R + h * 512,
                                       ci * R + (h + 1) * 512)
                            nc.tensor.matmul(
                                psy[:, fs], wcs[h], spk,
                                start=True, stop=False,
                            )
                            nc.tensor.matmul(
                                psy[:, fs], wd2,
                                xg[:, co * R + h * 512:co * R + (h + 1) * 512],
                                start=False, stop=True,
                            )
                    og = ogp.tile([128, 2048], f16, tag="og",
                                  name=f"og{g}_{half}")
                    nc.scalar.activation(
                        og[:], psy[:],
                        mybir.ActivationFunctionType.Tanh,
                    )
                    oq = oqp.tile([128, 2048], i8, tag="oq",
                                  name=f"oq{g}_{half}")
                    nc.vector.tensor_scalar_mul(oq[:], og[:], 127.0)
                    nc.sync.dma_start(
                        yv[g][:, half * 2048:(half + 1) * 2048], oq[:]
                    )

    nc.compile()
    return nc


def _get_nc():
    if "nc" not in _CACHE:
        _CACHE["nc"] = _build_nc()
    return _CACHE["nc"]


def _prepare_in_maps(inputs) -> list[dict]:
    """Full inputs -> per-core in_maps (host does transpose + fp16 cast)."""
    x = np.asarray(inputs["x"], dtype=np.float32)
    hw = np.asarray(inputs["hidden_weights"], dtype=np.float32)
    cw = np.asarray(inputs["communication_weights"], dtype=np.float32)
    assert x.shape == (BATCH, D), x.shape

    wc7t = cw.T / np.float32(NORM)          # [64, 64]
    wdt = hw.T - wc7t                       # [64, 64]
    wpk = np.zeros((128, 384), dtype=np.float16)
    wpk[0:64, 0:64] = wdt                   # wd2 block-diagonal
    wpk[64:128, 64:128] = wdt
    wpk[0:64, 128:192] = wc7t               # wcs_lo: weights in rows 0:64
    wpk[0:64, 192:256] = wc7t
    wpk[64:128, 256:320] = wc7t             # wcs_hi: weights in rows 64:128
    wpk[64:128, 320:384] = wc7t

    s = x.reshape(BATCH, NAGENT, DA).sum(axis=1)        # [B, 64] in f32
    x16 = x.astype(np.float16)
    s16 = s.astype(np.float16)

    in_maps = []
    for i in range(NCORES):
        rows = slice(i * SHARD, (i + 1) * SHARD)
        xt = x16[rows].T                                 # [512, 8192]
        st = s16[rows].T                                 # [64, 8192]
        # x cols: [4, 128, 8, 1024] -> [8, 128, 4, 1024]
        x7 = np.empty((NGROUP, 128, GCOL), dtype=np.float16)
        x7[:, :, 0:XCOL] = (
            xt.reshape(NCHUNK, 128, NGROUP, R).transpose(2, 1, 0, 3)
            .reshape(NGROUP, 128, XCOL)
        )
        # s-pack: [g, h*64 + k, c] = s.T[k, g*1024 + h*512 + c]
        x7[:, :, XCOL:GCOL] = (
            st.reshape(DA, NGROUP, 2, 512).transpose(1, 2, 0, 3)
            .reshape(NGROUP, 128, SCOL)
        )
        in_maps.append({"x7": x7.reshape(NGROUP * 128, GCOL), "wpk": wpk})
    return in_maps


def _decode_out(res) -> np.ndarray:
    y = np.empty((BATCH, D), dtype=np.float32)
    inv = np.float32(1.0 / 127.0)
    for i, r in enumerate(res.results):
        y4 = r["y4"].reshape(NGROUP, 128, NCHUNK, R)
        # y4[g, p, co, r] = 127 * y[g*R + r, co*128 + p]
        yi = y4.transpose(0, 3, 2, 1).reshape(SHARD, D)
        y[i * SHARD:(i + 1) * SHARD] = yi
    y *= inv
    return y


def kernel(**inputs) -> np.ndarray:
    from concourse.bass_utils import run_bass_kernel_spmd

    nc = _get_nc()
    in_maps = _prepare_in_maps(inputs)
    res = run_bass_kernel_spmd(nc, in_maps, core_ids=list(range(NCORES)))
    return _decode_out(res)
